# revision 1
# baseline (speedup 1.0000x reference)
"""BEV conv stack v2: fp8-DoubleRow L1 + f16 L2-4, deinterleaved-parity psum,
relu/bias in pass1, x-pool as contiguous tensor_tensor, y-pool via partition
fold, per-core edge masks. Host does binning + fp8 quant + tile prep.

Per-core (core = 2b + h): computes out[b, :, 64h:64h+64, :].
Coordinate chain (local coords per core):
  L1 pre-pool rows: g1 = 512h-16 + [0, 544), 34 tiles x 16 rows.
  L1 pooled local r1 in [0,272),   global Pg1 = r1 + 256h - 8.
  L2 tiles u: rows r1 in [8u+1, 8u+11); out pre-pool r1 [8u+2, 8u+10).
  L2 pooled r2 = r1/2 in [1,137),  global Pg2 = r2 + 128h - 4.
  L3 tiles v: rows r2 in [4v+1, 4v+7); out r2 [4v+2, 4v+6).
  L3 pooled r3 in [1,67),          global Pg3 = r3 + 64h - 2.
  L4 tiles w: rows r3 in [2w+1, 2w+5); out r3 [2w+2, 2w+4) = global 64h+2w+{0,1}.
"""
import sys
sys.path.insert(0, '/opt/trn_rl_repo')
import numpy as np
import ml_dtypes

PR = [0.0, -39.68, -3.0, 69.12, 39.68, 1.0]
W = 1024
H = 1024
B = 4
BN_EPS = 1e-5
F8MAX = 240.0
_CACHE = {}

f8np = ml_dtypes.float8_e4m3


# ---------------- host reference-faithful binning ----------------
def _bin_points(points):
    pts = np.asarray(points, dtype=np.float32)
    xs = np.float32(W / (PR[3] - PR[0]))
    ys = np.float32(H / (PR[4] - PR[1]))
    half = np.float32((PR[4] - PR[1]) / 2)
    xp = (pts[:, 1] * xs).astype(np.int32)
    yp = ((pts[:, 2] + half) * ys).astype(np.int32)
    b = pts[:, 0].astype(np.int32)
    mask = (xp >= 0) & (xp < W) & (yp >= 0) & (yp < H)
    lin = (b * H + yp) * W + xp
    z = pts[:, 3]
    inten = pts[:, 4]
    n = B * H * W
    lv = lin[mask]
    cnt = np.bincount(lv, minlength=n).astype(np.float32)
    zmin = np.full(n, 10.0, np.float32)
    np.minimum.at(zmin, lv, z[mask])
    zmax = np.full(n, -10.0, np.float32)
    np.maximum.at(zmax, lv, z[mask])
    iv = np.zeros(n, np.float32)
    np.maximum.at(iv, lv, inten[mask])
    bev0 = np.where(cnt == 0, np.float32(1.0), cnt) / np.float32(50.0)
    grids = np.stack([bev0, zmin, zmax, iv], axis=0).reshape(4, B, H, W)
    return np.transpose(grids, (1, 0, 2, 3))  # [B,4,H,W]


def _fold(w, b, g, be, m, v):
    sc = np.asarray(g, np.float32) / np.sqrt(np.asarray(v, np.float32) + BN_EPS)
    wf = np.asarray(w, np.float32) * sc[:, None, None, None]
    bf = (np.asarray(b, np.float32) - np.asarray(m, np.float32)) * sc + np.asarray(be, np.float32)
    return wf, bf


# ---------------- layer M layouts ----------------
# L1: m = ypar*64 + yq*8 + co   (y = 2yq+ypar in [0,16), co in [0,8))
# L2: m = ypar*64 + yq*16 + co  (y in [0,8), co 16)
# L3: m = ypar*64 + yq*32 + co  (y in [0,4), co 32)
# L4: m = y*64 + co             (y in [0,2), co 64)

def _build_lhst1(wq, chunk):
    """wq [2(hi/lo), 8co, 4c, 3dy, 3dx] f32 (already /sw, quantized values).
    -> [108, 2, 128] f8 for chunk; tap T = dx*72 + r*4 + c, k = T - 108*chunk.
    entry[k, hl, m] = wq[hl, co, c, dy=r-y, dx]."""
    out = np.zeros((108, 2, 128), np.float32)
    for k in range(108):
        T = chunk * 108 + k
        dx, rem = divmod(T, 72)
        r, c = divmod(rem, 4)
        for y in range(16):
            dy = r - y
            if 0 <= dy < 3:
                m = (y % 2) * 64 + (y // 2) * 8 + np.arange(8)
                out[k, :, m] = wq[:, :, c, dy, dx].T  # hmm shape check below
    return out


def _build_lhst1_v(wq):
    """Vectorized: wq [2, 8, 4, 3, 3] -> two [108, 2, 128] f32."""
    full = np.zeros((216, 2, 128), np.float32)
    for dx in range(3):
        for r in range(18):
            for c in range(4):
                T = dx * 72 + r * 4 + c
                for y in range(max(0, r - 2), min(16, r + 1)):
                    dy = r - y
                    mbase = (y % 2) * 64 + (y // 2) * 8
                    full[T, :, mbase:mbase + 8] = wq[:, :, c, dy, dx]
    return full[:108], full[108:]


def _build_lhst(wf, ci, co, ny, eta, nco_stride):
    """f16 layers: -> [eta*ci, 3, 128]; k = e*ci + c;
    m = ypar*64 + yq*nco_stride... uses layout fn below."""
    K = eta * ci
    out = np.zeros((K, 3, 128), np.float32)
    for e in range(eta):
        for c in range(ci):
            k = e * ci + c
            for y in range(ny):
                dy = e - y
                if 0 <= dy < 3:
                    for o in range(co):
                        if ny > 2:
                            m = (y % 2) * 64 + (y // 2) * co + o
                        else:
                            m = y * 64 + o
                        out[k, :, m] = wf[o, c, dy, :]
    return out


def _prep_weights(inputs):
    """Returns dict of device weight arrays + scale/bias consts (shared all cores)."""
    w1f, b1f = _fold(inputs['w1'], inputs['b1'], inputs['g1'], inputs['be1'], inputs['m1'], inputs['v1'])
    w2f, b2f = _fold(inputs['w2'], inputs['b2'], inputs['g2'], inputs['be2'], inputs['m2'], inputs['v2'])
    w3f, b3f = _fold(inputs['w3'], inputs['b3'], inputs['g3'], inputs['be3'], inputs['m3'], inputs['v3'])
    w4f, b4f = _fold(inputs['w4'], inputs['b4'], inputs['g4'], inputs['be4'], inputs['m4'], inputs['v4'])

    bev_sc = _CACHE['bev_sc']  # [4] per-channel act scales
    w1p = w1f * bev_sc[None, :, None, None]  # absorb act scale
    sw = np.abs(w1p).max(axis=(1, 2, 3)) / 240.0 + 1e-30  # [8]
    w1n = w1p / sw[:, None, None, None]
    w1h = np.clip(w1n, -F8MAX, F8MAX).astype(f8np).astype(np.float32)
    w1l = np.clip(w1n - w1h, -F8MAX, F8MAX).astype(f8np).astype(np.float32)
    wq = np.stack([w1h, w1l], axis=0)  # [2, 8, 4, 3, 3]
    lA, lB = _build_lhst1_v(wq)
    d = {}
    d['w1a'] = lA.astype(f8np).reshape(108, 256)
    d['w1b'] = lB.astype(f8np).reshape(108, 256)
    l2w = np.zeros((88, 3, 128), np.float32)
    l2w[8:88] = _build_lhst(w2f, 8, 16, 8, 10, 16)
    d['w2'] = l2w.astype(np.float16).reshape(88, 384)
    d['w3'] = _build_lhst(w3f, 16, 32, 4, 6, 32).astype(np.float16).reshape(96, 384)
    w4full = _build_lhst(w4f, 32, 64, 2, 4, 64).astype(np.float16)
    d['w4a'] = w4full[0:64].reshape(64, 384)
    d['w4b'] = w4full[64:128].reshape(64, 384)

    # consts [128, 5]: sc1, b1, b2, b3, b4 per-partition
    co_of = {}
    cons = np.zeros((128, 5), np.float32)
    m = np.arange(128)
    cons[:, 0] = sw[(m % 64) % 8]              # L1 scale per m (co = m%8 within ypar blocks)
    cons[:, 1] = b1f[(m % 64) % 8]
    cons[:, 2] = b2f[(m % 64) % 16]
    cons[:, 3] = b3f[(m % 64) % 32]
    cons[:, 4] = b4f[m % 64]
    d['cons'] = cons
    return d


def _masks_for_core(h):
    """[128, 7] f32: slots L2t0, L2t32, L2t33, L3t0, L3t32, L4t0, L4t31.
    partition p = e*ci + c for tile layouts."""
    mk = np.ones((128, 7), np.float32)

    def setm(slot, eta, ci, valid_fn):
        for e in range(eta):
            v = 1.0 if valid_fn(e) else 0.0
            mk[e * ci:(e + 1) * ci, slot] = v
    # L2 tile u: part p -> row 8u + p//8, Pg1 = row + 256h - 8, valid < 512
    setm(0, 11, 8, lambda e, u=0: 0 <= (8 * u + e) + 256 * h - 8 < 512)
    setm(1, 11, 8, lambda e, u=32: 0 <= (8 * u + e) + 256 * h - 8 < 512)
    setm(2, 11, 8, lambda e, u=33: 0 <= (8 * u + e) + 256 * h - 8 < 512)
    # L3 tile v: part p -> row 4v+1 + p//16, Pg2 = row + 128h - 4, valid < 256
    setm(3, 6, 16, lambda e, v=0: 0 <= (4 * v + 1 + e) + 128 * h - 4 < 256)
    setm(4, 6, 16, lambda e, v=32: 0 <= (4 * v + 1 + e) + 128 * h - 4 < 256)
    # L4 slab tile s: part p -> row 2s+1 + p//32, Pg3 = row + 64h - 2, valid < 128
    setm(5, 2, 32, lambda e, s=0: 0 <= (2 * s + 1 + e) + 64 * h - 2 < 128)
    setm(6, 2, 32, lambda e, s=32: 0 <= (2 * s + 1 + e) + 64 * h - 2 < 128)
    return mk


def _build_bev_tiles(grid_b, h):
    """grid_b [4, 1024, 1024] f32 -> quantized dx-folded chunk tiles
    [2, 34, 108, 1024] f8. tap T = dx*72 + r*4 + c; tile t input rows
    global g1+16t-1+r, g1 = 512h-16; col j of tile = input col j+dx-1."""
    bev_sc = _CACHE['bev_sc']
    q = np.clip(grid_b / bev_sc[:, None, None], -F8MAX, F8MAX).astype(f8np)
    g1 = 512 * h - 16
    # padded rows: global rows [g1-1, g1+545) -> 546 rows; x pad 1 both sides
    padded = np.zeros((4, 546, 1026), f8np)
    lo = max(0, g1 - 1)
    hi = min(1024, g1 + 545)
    padded[:, lo - (g1 - 1):hi - (g1 - 1), 1:1025] = q[:, lo:hi, :]
    pf = padded  # [c, R, 1026]; tile t row r -> R-index 16t + r; col j tap dx -> 1026-index j + dx
    out = np.zeros((2, 108, 34, 1024), f8np)
    # build [T, t, col]: for each (dx, r, c): row slice [16t+r], cols [dx : dx+1024]
    for dx in range(3):
        for r in range(18):
            rows = pf[:, r:r + 16 * 34:16, dx:dx + 1024]  # [4, 34, 1024]
            for c in range(4):
                T = dx * 72 + r * 4 + c
                ch, k = divmod(T, 108)
                out[ch, k] = rows[c]
    return out.reshape(2, 108, 34 * 1024)


# ---------------- bass module ----------------
def _build_module():
    import concourse.mybir as mybir
    from concourse.tile import TileContext
    from concourse import bacc

    f32 = mybir.dt.float32
    f16 = mybir.dt.float16
    f8 = mybir.dt.float8e4
    AL = mybir.AluOpType
    RELU = mybir.ActivationFunctionType.Relu
    DR = mybir.MatmulPerfMode.DoubleRow

    nc = bacc.Bacc()
    bev = nc.dram_tensor("bev", [2, 108, 34 * 1024], f8, kind="ExternalInput")
    w1a = nc.dram_tensor("w1a", [108, 256], f8, kind="ExternalInput")
    w1b = nc.dram_tensor("w1b", [108, 256], f8, kind="ExternalInput")
    w2 = nc.dram_tensor("w2", [88, 384], f16, kind="ExternalInput")
    w3 = nc.dram_tensor("w3", [96, 384], f16, kind="ExternalInput")
    w4a = nc.dram_tensor("w4a", [64, 384], f16, kind="ExternalInput")
    w4b = nc.dram_tensor("w4b", [64, 384], f16, kind="ExternalInput")
    cons = nc.dram_tensor("cons", [128, 5], f32, kind="ExternalInput")
    msk = nc.dram_tensor("msk", [128, 7], f32, kind="ExternalInput")
    out_d = nc.dram_tensor("out", [128, 4096], f32, kind="ExternalOutput")


    NQ = 9  # bev quads (4 tiles each, last has 2)

    with TileContext(nc) as tc:
        with tc.tile_pool(name="const", bufs=1) as cp, \
             tc.tile_pool(name="bevp", bufs=3) as bp, \
             tc.tile_pool(name="l2p", bufs=4) as l2p, \
             tc.tile_pool(name="l3p", bufs=4) as l3p, \
             tc.tile_pool(name="l4p", bufs=4) as l4p, \
             tc.tile_pool(name="work", bufs=3) as wp, \
             tc.tile_pool(name="stg", bufs=1) as sp, \
             tc.tile_pool(name="psum", bufs=1, space="PSUM") as pp:

            # ---- consts (issue from act/vector queues to spread SEQ load) ----
            tw1a = cp.tile([108, 256], f8, tag="w1a")
            tw1b = cp.tile([108, 256], f8, tag="w1b")
            tw2 = cp.tile([88, 384], f16, tag="w2")
            tw3 = cp.tile([96, 384], f16, tag="w3")
            tw4a = cp.tile([64, 384], f16, tag="w4a")
            tw4b = cp.tile([64, 384], f16, tag="w4b")
            tcons = cp.tile([128, 5], f32, tag="cons")
            tmsk = cp.tile([128, 7], f32, tag="msk")
            nc.sync.dma_start(out=tw1a[:], in_=w1a[:])
            nc.sync.dma_start(out=tw1b[:], in_=w1b[:])
            nc.sync.dma_start(out=tw2[:], in_=w2[:])
            nc.sync.dma_start(out=tw3[:], in_=w3[:])
            nc.sync.dma_start(out=tw4a[:], in_=w4a[:])
            nc.sync.dma_start(out=tw4b[:], in_=w4b[:])
            nc.sync.dma_start(out=tcons[:], in_=cons[:])
            nc.sync.dma_start(out=tmsk[:], in_=msk[:])
            SC1, B1, B2, B3, B4 = (tcons[:, i:i + 1] for i in range(5))

            # ---- tiles ----
            l2t = [l2p.tile([88, 514], f16, tag=f"a{u % 5}", name=f"l2_{u}", bufs=1) for u in range(34)]
            l3t = [l3p.tile([96, 258], f16, tag=f"a{v % 5}", name=f"l3_{v}", bufs=1) for v in range(33)]
            l4t = [l4p.tile([64, 130], f16, tag=f"a{w % 5}", name=f"l4_{w}", bufs=1) for w in range(33)]
            # zero first-use buffers (recycled buffers stay finite afterwards)
            for t in (l2t[:5] + l3t[:5] + l4t[:5]):
                nc.gpsimd.memset(t[:].bitcast(f32), 0.0)

            stg = sp.tile([128, 4096], f32, tag="stg")

            def bev_quad(q):
                n = 4 if q < 8 else 2
                ta = bp.tile([108, 4096], f8, tag="bqa", name=f"bqa{q}")
                tb = bp.tile([108, 4096], f8, tag="bqb", name=f"bqb{q}")
                nc.sync.dma_start(out=ta[:, 0:n * 1024],
                                  in_=bev[0][:, 4096 * q:4096 * q + n * 1024])
                nc.sync.dma_start(out=tb[:, 0:n * 1024],
                                  in_=bev[1][:, 4096 * q:4096 * q + n * 1024])
                return ta, tb

            bq = {}
            bq[0] = bev_quad(0)

            def l1_tile(t):
                if t % 4 == 0 and t // 4 + 1 < NQ:
                    bq[t // 4 + 1] = bev_quad(t // 4 + 1)
                ta, tb = bq[t // 4]
                off = (t % 4) * 1024
                ps = pp.tile([128, 1024], f32, tag="ps1", name=f"ps1_{t}", bufs=2)
                wva = tw1a[:].rearrange("p (a b) -> p a b", a=2)
                wvb = tw1b[:].rearrange("p (a b) -> p a b", a=2)
                for par in range(2):
                    for ci, (wt, bt) in enumerate(((wva, ta), (wvb, tb))):
                        # [108, 512] stride-2 view at parity offset
                        rv = bt[:, off:off + 1024].rearrange("p (xh two) -> p two xh", two=2)[:, par, :]
                        rv = rv.unsqueeze(1).broadcast_to((108, 2, 512))
                        nc.tensor.matmul(out=ps[:, par * 512:(par + 1) * 512],
                                         lhsT=wt, rhs=rv,
                                         start=(ci == 0), stop=(ci == 1), perf_mode=DR)
                # pass1: relu(ps*sc1 + b1) -> A f16 (ACT)
                A = wp.tile([128, 1024], f16, tag="A1", name=f"A1_{t}")
                nc.scalar.activation(out=A[:], in_=ps[:], func=RELU, bias=B1, scale=SC1)
                # x-pool (DVE): separate base-0 tiles per row-parity half
                Xe = wp.tile([64, 512], f16, tag="X1e", name=f"X1e_{t}")
                Xo = wp.tile([64, 512], f16, tag="X1o", name=f"X1o_{t}")
                nc.vector.tensor_tensor(out=Xe[:], in0=A[0:64, 0:512], in1=A[0:64, 512:1024], op=AL.max)
                nc.vector.tensor_tensor(out=Xo[:], in0=A[64:128, 0:512], in1=A[64:128, 512:1024], op=AL.max)
                # y-pool main: full slab -> l2t[t][0:64]; dup: DMA copy of
                # next tile's first 3 rows into [64:88] (ACT hwdge queue)
                nc.vector.tensor_tensor(out=l2t[t][0:64, 1:513], in0=Xe[:], in1=Xo[:], op=AL.max)
                if t >= 1:
                    nc.sync.dma_start(out=l2t[t - 1][64:88, 1:513], in_=l2t[t][0:24, 1:513])

            def mask_op(tile, np_, slot):
                nc.vector.tensor_scalar(out=tile[0:np_, :], in0=tile[0:np_, :],
                                        scalar1=tmsk[0:np_, slot:slot + 1], scalar2=None,
                                        op0=AL.mult)

            def l2_tile(u):
                ps = pp.tile([128, 512], f32, tag="ps2", name=f"ps2_{u}", bufs=2)
                for dx in range(3):
                    rv = l2t[u][0:88, dx:dx + 512].rearrange("p (xh two) -> p two xh", two=2)
                    nc.tensor.matmul(out=ps[:], lhsT=tw2[:, dx * 128:(dx + 1) * 128],
                                     rhs=rv, start=(dx == 0), stop=(dx == 2))
                A = wp.tile([128, 512], f16, tag="A2", name=f"A2_{u}")
                nc.scalar.activation(out=A[:], in_=ps[:], func=RELU, bias=B2, scale=1.0)
                Xe = wp.tile([64, 256], f16, tag="X2e", name=f"X2e_{u}")
                Xo = wp.tile([64, 256], f16, tag="X2o", name=f"X2o_{u}")
                nc.vector.tensor_tensor(out=Xe[:], in0=A[0:64, 0:256], in1=A[0:64, 256:512], op=AL.max)
                nc.vector.tensor_tensor(out=Xo[:], in0=A[64:128, 0:256], in1=A[64:128, 256:512], op=AL.max)
                if u < 33:
                    nc.vector.tensor_tensor(out=l3t[u][0:64, 1:257], in0=Xe[:], in1=Xo[:], op=AL.max)
                if u >= 1 and u - 1 < 33:
                    if u < 33:
                        nc.vector.tensor_copy(out=l3t[u - 1][64:96, 1:257], in_=l3t[u][0:32, 1:257])
                    else:
                        nc.vector.tensor_tensor(out=l3t[u - 1][64:96, 1:257], in0=Xe[0:32, :], in1=Xo[0:32, :], op=AL.max)

            def l3_tile(v):
                ps = pp.tile([128, 256], f32, tag="ps3", name=f"ps3_{v}", bufs=1)
                for dx in range(3):
                    rv = l3t[v][0:96, dx:dx + 256].rearrange("p (xh two) -> p two xh", two=2)
                    nc.tensor.matmul(out=ps[:], lhsT=tw3[:, dx * 128:(dx + 1) * 128],
                                     rhs=rv, start=(dx == 0), stop=(dx == 2))
                A = wp.tile([128, 256], f16, tag="A3", name=f"A3_{v}")
                nc.scalar.activation(out=A[:], in_=ps[:], func=RELU, bias=B3, scale=1.0)
                Xe = wp.tile([64, 128], f16, tag="X3e", name=f"X3e_{v}")
                Xo = wp.tile([64, 128], f16, tag="X3o", name=f"X3o_{v}")
                nc.vector.tensor_tensor(out=Xe[:], in0=A[0:64, 0:128], in1=A[0:64, 128:256], op=AL.max)
                nc.vector.tensor_tensor(out=Xo[:], in0=A[64:128, 0:128], in1=A[64:128, 128:256], op=AL.max)
                # slab write only; l4 conv reads tiles w and w+1 via K-split
                nc.vector.tensor_tensor(out=l4t[v][0:64, 1:129], in0=Xe[:], in1=Xo[:], op=AL.max)

            def l4_tile(w):
                ps = pp.tile([128, 128], f32, tag="ps4", name=f"ps4_{w}", bufs=1)
                for dx in range(3):
                    nc.tensor.matmul(out=ps[:], lhsT=tw4a[:, dx * 128:(dx + 1) * 128],
                                     rhs=l4t[w][0:64, dx:dx + 128], start=(dx == 0), stop=False)
                    nc.tensor.matmul(out=ps[:], lhsT=tw4b[:, dx * 128:(dx + 1) * 128],
                                     rhs=l4t[w + 1][0:64, dx:dx + 128], start=False, stop=(dx == 2))
                if w % 2 == 0:
                    nc.vector.tensor_scalar(out=stg[:, w * 128:(w + 1) * 128], in0=ps[:],
                                            scalar1=B4, scalar2=0.0, op0=AL.add, op1=AL.max)
                else:
                    nc.scalar.activation(out=stg[:, w * 128:(w + 1) * 128], in_=ps[:],
                                         func=RELU, bias=B4, scale=1.0)
                if w % 8 == 7:
                    nc.sync.dma_start(out=out_d[:, (w - 7) * 128:(w + 1) * 128],
                                      in_=stg[:, (w - 7) * 128:(w + 1) * 128])

            import os
            STAGE = int(os.environ.get('V2STAGE', '4'))
            if STAGE < 4:
                nc.vector.memset(stg[:, 0:4096], 0.0)
                nc.sync.dma_start(out=out_d[:], in_=stg[:])
            for i in range(41):
                if i < 34:
                    l1_tile(i)
                    if i == 1:
                        mask_op(l2t[0], 88, 0)
                    if i == 33:
                        mask_op(l2t[32], 88, 1)
                        mask_op(l2t[33], 88, 2)
                if 4 <= i and STAGE >= 2:
                    u = i - 4
                    if u < 34:
                        l2_tile(u)
                        if u == 1:
                            mask_op(l3t[0], 96, 3)
                        if u == 33:
                            mask_op(l3t[32], 96, 4)
                if 6 <= i and STAGE >= 3:
                    v = i - 6
                    if v < 33:
                        l3_tile(v)
                        if v == 0:
                            mask_op(l4t[0], 64, 5)
                        if v == 32:
                            mask_op(l4t[32], 64, 6)
                if 8 <= i and STAGE >= 4:
                    w = i - 8
                    if w < 32:
                        l4_tile(w)

    nc.finalize()
    return nc


# ---------------- entry ----------------
def kernel(points, batch_size,
           w1, b1, g1, be1, m1, v1,
           w2, b2, g2, be2, m2, v2,
           w3, b3, g3, be3, m3, v3,
           w4, b4, g4, be4, m4, v4, **_):
    from concourse.bass_utils import run_bass_kernel_spmd

    grids = _bin_points(points)  # [4,4,1024,1024]
    _CACHE['bev_sc'] = np.abs(grids).max(axis=(0, 2, 3)).astype(np.float32) / 240.0 + 1e-30

    inputs = dict(w1=w1, b1=b1, g1=g1, be1=be1, m1=m1, v1=v1,
                  w2=w2, b2=b2, g2=g2, be2=be2, m2=m2, v2=v2,
                  w3=w3, b3=b3, g3=g3, be3=be3, m3=m3, v3=v3,
                  w4=w4, b4=b4, g4=g4, be4=be4, m4=m4, v4=v4)
    wd = _prep_weights(inputs)

    core_ids = list(range(8))
    in_maps = []
    for core in core_ids:
        b, h = core // 2, core % 2
        im = dict(wd)
        im['bev'] = _build_bev_tiles(grids[b], h)
        im['msk'] = _masks_for_core(h)
        in_maps.append(im)

    if 'nc' not in _CACHE:
        _CACHE['nc'] = _build_module()
    nc = _CACHE['nc']
    r = run_bass_kernel_spmd(nc, in_maps, core_ids=core_ids)

    out_full = np.zeros((B, 64, 128, 128), np.float32)
    for i, core in enumerate(core_ids):
        b, h = core // 2, core % 2
        S = r.results[i]["out"]  # [128, 4096]
        # S[y*64+co, w*128+x] = out[b, co, 64h+2w+y, x]
        Sv = S.reshape(2, 64, 32, 128)          # [y, co, w, x]
        out_full[b, :, 64 * h:64 * h + 64, :] = np.transpose(Sv, (1, 2, 0, 3)).reshape(64, 64, 128)
    return out_full

